# revision 21
# baseline (speedup 1.0000x reference)
"""3-layer GCN (message passing) + sum-pool + MLP head on 8 Trainium2 cores.

Strategy (all shapes hardcoded; self-contained):
  - Host graph preprocessing: permute nodes into 392 blocks of 128. The
    global table row order is (chunk, core, block): 4 chunk segments of
    15/10/15/9 local blocks per core, so each segment is exactly one
    rank-major AllGather output.
  - Per-block edge slots are split FOUR ways by the source node's chunk
    segment (caps 640/512/640/384 = 17 chunks of 128), so each gather
    call only depends on ONE AllGather chunk. AG chunk k fires as soon
    as its producing batches are done (after batch 2/4/7; the last chunk
    is deferred into the consuming layer, after its first lookahead
    gathers). Gather issue runs as a wavefront: source-chunk-0/1 calls
    lead source-chunk-2/3 calls, so the consuming layer's early gathers
    start while the producer's tail and the late AG chunks are still in
    flight.
  - Layer 1 aggregates x*d_inv directly (linearity: A(xW0) = (Ax)W0), so
    the L1 gather table is a replicated input - no AllGather.
  - Tables are bf16 [*, 128]. One 0/1 one-hot routing table (shared by
    all three layers) is HOST-precomputed and streamed in by DMA. GCN
    normalization folds into d_inv^2 epilogues for L1/L2 and a
    per-column d_inv scale before L3's relu (commutes since d_inv > 0).
    Self-loops are a constant identity-matmul chunk.
  - Gather calls round-robin the 4 SWDGE queues; AllGather outputs live
    in Shared DRAM; pooled vector is AllReduce'd; tiny MLP head runs
    replicated.
"""
import sys

import numpy as np

for _p in ("/opt/trn_rl_repo", "/root/.axon_site/_ro/trn_rl_repo"):
    if _p not in sys.path:
        sys.path.append(_p)

import ml_dtypes

import concourse.bacc as bacc
import concourse.bass as bass
import concourse.mybir as mybir
import concourse.tile as tile
from concourse.bass_utils import run_bass_kernel_spmd

# ---------------------------------------------------------------- constants
N = 50000                 # real nodes
P = 128
NCORES = 8
BPC = 49                  # blocks per core
NB = BPC * NCORES         # 392 blocks
NP = NB * P               # padded nodes = 50176
ROWS_PC = BPC * P         # 6272 rows per core shard
CH_BLK = [0, 15, 25, 40, 49]          # AllGather chunk bounds (local blocks)
SEG_BASE = [0, 15360, 25600, 40960, 50176]  # global row base per chunk
NCHUNK = 4
CAPS = (640, 512, 640, 384)  # per-block edge caps by source chunk
GCH = (5, 4, 5, 3)           # = CAPS/128: matmul chunks per source group
COFF = (0, 5, 9, 14)         # one-hot column offset of each group's chunks
CHB = sum(GCH)               # 17 edge chunks per block
CAPB_TOT = sum(CAPS)         # 2176 edge slots per block
BATCH_SIZES = [5] * 9 + [4]  # gather batching of the 49 blocks
NT = len(BATCH_SIZES)
PRE_E = 3                 # lookahead (batches) for source groups 0/1
PRE_L = 2                 # lookahead for source groups 2/3
AG_FIRE = {2: 0, 4: 1, 7: 2}  # consume-batch -> AG chunk to fire
IDX_COLS = (CAPB_TOT // 16) * BPC  # 6664 idx columns (int16, wrap 16)
FW = 128                  # stored table width (bf16)

_CACHED_NC = None
BF16 = ml_dtypes.bfloat16


# ------------------------------------------------------------- host prepro
def _balance_blocks4(loads, nblocks, caps):
    """Greedy-pack nodes (per-node 4-dim in-loads) into blocks of <=128
    nodes with per-group loads <= caps. Returns block id per node row."""
    order = np.argsort(-loads.sum(1), kind="stable")
    la = np.zeros((nblocks, 4), np.int64)
    cnt = np.zeros(nblocks, np.int64)
    capv = np.array(caps, np.float64)
    out = np.empty(len(loads), np.int64)
    for i in order:
        na = la + loads[i]
        score = (na / capv).max(1)
        score[(cnt >= P) | (na > capv).any(1)] = np.inf
        j = int(np.argmin(score))
        assert np.isfinite(score[j]), "block packing infeasible; raise CAPS"
        out[i] = j
        la[j] = na[j]
        cnt[j] += 1
    return out


def _cj_to_row(c, j):
    """(core, local block) -> global table row base (numpy-friendly)."""
    k = np.searchsorted(np.array(CH_BLK), j, side="right") - 1
    w = np.array([CH_BLK[i + 1] - CH_BLK[i] for i in range(NCHUNK)])
    base = np.array(SEG_BASE[:NCHUNK])
    return base[k] + (c * w[k] + (j - np.array(CH_BLK)[k])) * P


def _preprocess(x, edge_index):
    src = np.asarray(edge_index[0], np.int64)
    dst = np.asarray(edge_index[1], np.int64)

    deg = np.bincount(dst, minlength=N).astype(np.float64)
    d_inv = 1.0 / np.sqrt(deg + 1.0)

    # ---- assign nodes to the 4 chunk groups, biasing out-edge mass to
    # match each group's share of per-block edge capacity
    gslots = np.array([(CH_BLK[g + 1] - CH_BLK[g]) * NCORES * P
                       for g in range(NCHUNK)])
    targ = np.array(CAPS, np.float64) / CAPB_TOT
    out_w = np.bincount(src, minlength=N)
    order = np.argsort(-out_w, kind="stable")
    grp = np.zeros(N, np.int8)
    tot = np.zeros(NCHUNK)
    cnti = np.zeros(NCHUNK, np.int64)
    for i in order:
        frac = tot / targ
        frac[cnti >= gslots] = np.inf
        g = int(np.argmin(frac))
        grp[i] = g
        tot[g] += out_w[i]
        cnti[g] += 1

    # ---- per-node in-loads split by source group
    sg = grp[src]
    loads = np.zeros((N, NCHUNK), np.int64)
    for g in range(NCHUNK):
        loads[:, g] = np.bincount(dst[sg == g], minlength=N)

    # ---- pack each group's nodes into its blocks; round-robin blocks
    # over cores (group g -> local blocks CH_BLK[g]..CH_BLK[g+1]-1)
    perm_pos = np.empty(N, np.int64)  # node -> global table row
    for g in range(NCHUNK):
        nodes = np.nonzero(grp == g)[0]
        nblocks = (CH_BLK[g + 1] - CH_BLK[g]) * NCORES
        blk = _balance_blocks4(loads[nodes], nblocks, CAPS)
        o2 = np.argsort(blk, kind="stable")
        sb = blk[o2]
        grp_start = np.searchsorted(sb, np.arange(nblocks), side="left")
        pos_in_grp = np.arange(len(nodes)) - grp_start[sb]
        core = sb % NCORES
        jloc = CH_BLK[g] + sb // NCORES
        perm_pos[nodes[o2]] = _cj_to_row(core, jloc) + pos_in_grp

    # ---- remap edges; (core, local block) of each dst row
    psrc = perm_pos[src]
    pdst = perm_pos[dst]

    w_arr = np.array([CH_BLK[i + 1] - CH_BLK[i] for i in range(NCHUNK)])
    seg = np.searchsorted(np.array(SEG_BASE), pdst, side="right") - 1
    r_in_seg = pdst - np.array(SEG_BASE)[seg]
    dc = r_in_seg // (w_arr[seg] * P)
    dj = np.array(CH_BLK)[seg] + (r_in_seg % (w_arr[seg] * P)) // P
    es = pdst % P               # dst slot
    ge = np.searchsorted(np.array(SEG_BASE), psrc, side="right") - 1
    eidx = psrc - np.array(SEG_BASE)[ge]   # gather idx within source group

    key = (dc * BPC + dj) * NCHUNK + ge
    order_e = np.argsort(key, kind="stable")
    key_s = key[order_e]
    cnts = np.bincount(key_s, minlength=NB * NCHUNK)
    cap_arr = np.tile(np.array(CAPS), NB)
    assert (cnts <= cap_arr).all(), "block-group overflow; raise CAPS"
    starts = np.concatenate([[0], np.cumsum(cnts)[:-1]])
    pos = np.arange(len(key_s)) - starts[key_s]

    # ---- fill per-core device arrays
    idxs = np.zeros((NCORES, 16, IDX_COLS), np.int16)
    oh1 = np.zeros((NCORES, P, BPC * CHB, P), ml_dtypes.float8_e4m3fn)

    g_core = dc[order_e]
    g_j = dj[order_e]
    g_g = ge[order_e]
    bs_arr = np.array(BATCH_SIZES)
    blk2batch = np.repeat(np.arange(NT), bs_arr)
    batch_blk0 = np.concatenate([[0], np.cumsum(bs_arr)[:-1]])
    g_t = blk2batch[g_j]
    g_k = g_j - batch_blk0[g_t]           # block within batch

    # one-hot: col = j*17 + COFF[g] + pos//128, row = pos%128, val col es
    col_dw = g_j * CHB + np.array(COFF)[g_g] + pos // P
    oh1[g_core, pos % P, col_dw, es[order_e]] = 1

    # idx: batch-grouped wrapped layout; per batch the 4 group calls are
    # laid out back to back (g0|g1|g2|g3), each (CAPS[g]//16)*bs cols
    batch_col0 = np.concatenate(
        [[0], np.cumsum((CAPB_TOT // 16) * bs_arr)[:-1]])
    goff16 = np.concatenate([[0], np.cumsum(np.array(CAPS) // 16)[:-1]])
    call_off = batch_col0[g_t] + goff16[g_g] * bs_arr[g_t]
    q = g_k * np.array(CAPS)[g_g] + pos
    idxs[g_core, q % 16, call_off + q // 16] = eidx[order_e]
    idxs_full = np.tile(idxs, (1, 8, 1))  # replicate to 128 partitions

    # ---- bf16 L1 gather table: xg[perm(n), 0:14] = x[n] * d_inv[n]
    xg = np.zeros((NP, FW), BF16)
    xg[perm_pos, :14] = (np.asarray(x, np.float64)
                         * d_inv[:, None]).astype(BF16)
    # per-core shard: its rows from each of the 4 chunk segments
    xg_own = np.concatenate([
        xg[SEG_BASE[k]:SEG_BASE[k + 1]].reshape(
            NCORES, (CH_BLK[k + 1] - CH_BLK[k]) * P, FW)
        for k in range(NCHUNK)], axis=1)

    # ---- per-slot d_inv arrays
    dinv2 = np.zeros((NCORES, P, BPC), np.float32)   # [core][slot, block]
    seg_n = np.searchsorted(np.array(SEG_BASE), perm_pos, side="right") - 1
    r_n = perm_pos - np.array(SEG_BASE)[seg_n]
    nc_ = r_n // (w_arr[seg_n] * P)
    nj = np.array(CH_BLK)[seg_n] + (r_n % (w_arr[seg_n] * P)) // P
    dinv2[nc_, perm_pos % P, nj] = d_inv * d_inv
    # [core][feat(32), block*128 + slot] broadcast table of d_inv for L3
    dinvb = np.zeros((NCORES, BPC * P), np.float32)
    dinvb[nc_, nj * P + perm_pos % P] = d_inv
    dinvb = np.repeat(dinvb[:, None, :], 32, axis=1)
    return xg, xg_own, idxs_full, oh1, dinv2, dinvb


# ------------------------------------------------------------ device build
def _build_kernel():
    nc = bacc.Bacc("TRN2", target_bir_lowering=False, debug=False,
                   num_swdge_queues=4)
    dt = mybir.dt

    xg = nc.dram_tensor("xg", [NP, FW], dt.bfloat16, kind="ExternalInput")
    xgo = nc.dram_tensor("xgo", [ROWS_PC, FW], dt.bfloat16, kind="ExternalInput")
    w0 = nc.dram_tensor("w0", [14, 128], dt.float32, kind="ExternalInput")
    w1 = nc.dram_tensor("w1", [128, 128], dt.float32, kind="ExternalInput")
    w2p = nc.dram_tensor("w2p", [128, FW], dt.float32, kind="ExternalInput")
    fc11w = nc.dram_tensor("fc11w", [32, 16], dt.float32, kind="ExternalInput")
    fc11b = nc.dram_tensor("fc11b", [16, 1], dt.float32, kind="ExternalInput")
    fc12w = nc.dram_tensor("fc12w", [16, 1], dt.float32, kind="ExternalInput")
    fc12b = nc.dram_tensor("fc12b", [1, 1], dt.float32, kind="ExternalInput")
    ident = nc.dram_tensor("ident", [P, P], dt.bfloat16, kind="ExternalInput")
    dinv2 = nc.dram_tensor("dinv2", [P, BPC], dt.float32, kind="ExternalInput")
    dinvb = nc.dram_tensor("dinvb", [32, BPC * P], dt.float32,
                           kind="ExternalInput")
    idxs = nc.dram_tensor("idxs", [P, IDX_COLS], dt.int16, kind="ExternalInput")
    oh1 = nc.dram_tensor("oh1", [P, BPC * CHB * P], dt.float8e4,
                         kind="ExternalInput")
    out = nc.dram_tensor("out", [1, 1], dt.float32, kind="ExternalOutput")

    bs_arr = np.array(BATCH_SIZES)
    batch_col0 = np.concatenate(
        [[0], np.cumsum((CAPB_TOT // 16) * bs_arr)[:-1]])
    goff16 = np.concatenate([[0], np.cumsum(np.array(CAPS) // 16)[:-1]])
    batch_blk0 = np.concatenate([[0], np.cumsum(bs_arr)[:-1]])

    with tile.TileContext(nc) as tc:
        with (
            tc.tile_pool(name="const", bufs=1) as cst,
            tc.tile_pool(name="g0", bufs=4) as gp0,
            tc.tile_pool(name="g1", bufs=4) as gp1,
            tc.tile_pool(name="g2", bufs=3) as gp2,
            tc.tile_pool(name="g3", bufs=3) as gp3,
            tc.tile_pool(name="gsp", bufs=2) as gsp,
            tc.tile_pool(name="oh", bufs=3) as ohp,
            tc.tile_pool(name="rl", bufs=3) as rlp,
            tc.tile_pool(name="st", bufs=2) as stp,
            tc.tile_pool(name="misc", bufs=1) as msc,
            tc.tile_pool(name="psA", bufs=2, space="PSUM") as psa,
            tc.tile_pool(name="psX", bufs=2, space="PSUM") as psx,
            tc.tile_pool(name="psD", bufs=2, space="PSUM") as psd,
            tc.tile_pool(name="psP", bufs=1, space="PSUM") as psp,
            tc.tile_pool(name="dram", bufs=1, space="DRAM") as drm,
        ):
            gpools = [gp0, gp1, gp2, gp3]
            # resident constants
            idxs_t = cst.tile([P, IDX_COLS], dt.int16)
            ident_t = cst.tile([P, P], dt.bfloat16)
            dinv2_t = cst.tile([P, BPC], dt.float32)
            dinvb_t = cst.tile([32, BPC * P], dt.float32)
            w0_t = cst.tile([14, 128], dt.float32)
            w1_t = cst.tile([128, 128], dt.float32)
            w2p_t = cst.tile([128, FW], dt.float32)
            fc11w_t = cst.tile([32, 16], dt.float32)
            fc11b_t = cst.tile([16, 1], dt.float32)
            fc12w_t = cst.tile([16, 1], dt.float32)
            fc12b_t = cst.tile([1, 1], dt.float32)
            for t_, d_ in (
                (ident_t, ident),
                (dinv2_t, dinv2), (dinvb_t, dinvb),
                (w0_t, w0), (w1_t, w1), (w2p_t, w2p),
                (fc11w_t, fc11w), (fc11b_t, fc11b), (fc12w_t, fc12w),
                (fc12b_t, fc12b),
            ):
                nc.sync.dma_start(t_[:], d_[:])
            # idx table loaded per-batch-column-slice so the first gather
            # only waits for its own slice, not the full 1.7MB
            ic_bounds = np.concatenate(
                [[0], np.cumsum((CAPB_TOT // 16) * bs_arr)])
            for t in range(NT):
                eng = nc.sync if t % 2 == 0 else nc.scalar
                eng.dma_start(
                    idxs_t[:, int(ic_bounds[t]) : int(ic_bounds[t + 1])],
                    idxs[:, int(ic_bounds[t]) : int(ic_bounds[t + 1])])

            # internal DRAM (bf16 tables); AllGather outputs are Shared,
            # one tile per chunk (a Shared tile allows a single writer)
            seg_rows = [SEG_BASE[k + 1] - SEG_BASE[k] for k in range(NCHUNK)]
            g2s_t = drm.tile([ROWS_PC, FW], dt.bfloat16)
            g2_c = [drm.tile([seg_rows[k], FW], dt.bfloat16,
                             addr_space="Shared", name=f"g2c{k}")
                    for k in range(NCHUNK)]
            g3s_t = drm.tile([ROWS_PC, FW], dt.bfloat16)
            g3_c = [drm.tile([seg_rows[k], FW], dt.bfloat16,
                             addr_space="Shared", name=f"g3c{k}")
                    for k in range(NCHUNK)]
            pool_in = drm.tile([32, 1], dt.float32)
            pool_out = drm.tile([32, 1], dt.float32, addr_space="Shared")

            pooled_cols = msc.tile([32, BPC], dt.float32)

            def ag_chunk(h_shard, h_chunks, k, w=FW):
                # collectives require contiguous APs, so chunks always move
                # full-width rows (w is accepted for call-site symmetry)
                nc.gpsimd.collective_compute(
                    "AllGather", mybir.AluOpType.bypass,
                    replica_groups=[list(range(NCORES))],
                    ins=[h_shard[CH_BLK[k] * P : CH_BLK[k + 1] * P, :].opt()],
                    outs=[h_chunks[k][:, :].opt()])

            def layer(lnum, h_srcs, h_self, h_shard, h_chunks, fire_prev,
                      ag_w=FW):
                gtiles = {}

                def emit(t, g):
                    # desc-gen serializes on the GpSimd engine (the ucode
                    # idles all non-target Q7 pairs per call), so call
                    # granularity/queue choice only affect overheads: use
                    # one call per (batch, group), rotating queues
                    bs = BATCH_SIZES[t]
                    tl = gpools[g].tile([P, GCH[g] * 5, FW], dt.bfloat16,
                                        tag=f"g{g}")
                    c0 = int(batch_col0[t] + goff16[g] * bs)
                    nc.gpsimd.dma_gather(
                        tl[:, : GCH[g] * bs, :], h_srcs[g],
                        idxs_t[:, c0 : c0 + (CAPS[g] // 16) * bs],
                        CAPS[g] * bs, CAPS[g] * bs, FW, single_packet=False,
                        queue_num=(t + g) % 4)
                    gtiles[(g, t)] = tl

                # wavefront prologue: source-chunk 0/1 gathers lead; the
                # previous boundary's last AG chunk fires after them (its
                # input is long ready; ordering keeps the Pool sequencer
                # from head-of-line blocking on it)
                for u in range(PRE_E):
                    emit(u, 0)
                    emit(u, 1)
                if fire_prev is not None:
                    fire_prev()
                for u in range(PRE_L):
                    emit(u, 2)
                    emit(u, 3)

                pending_ag = None
                for t, bs in enumerate(BATCH_SIZES):
                    if t + PRE_E < NT:
                        emit(t + PRE_E, 0)
                        emit(t + PRE_E, 1)
                    if t + PRE_L < NT:
                        emit(t + PRE_L, 2)
                        emit(t + PRE_L, 3)
                    # fire the AG whose inputs completed last batch AFTER
                    # this wave's gather dispatches: the trigger's wait
                    # would otherwise head-of-line block them on the Pool
                    # sequencer
                    if pending_ag is not None:
                        ag_chunk(h_shard, h_chunks, pending_ag, ag_w)
                        pending_ag = None
                    b0 = int(batch_blk0[t])
                    gs = gsp.tile([P, 5, FW], dt.bfloat16, tag="gs")
                    nc.sync.dma_start(
                        gs[:, :bs, :],
                        h_self[b0 * P : (b0 + bs) * P, :].rearrange(
                            "(g p) f -> p g f", p=P))
                    ohv = ohp.tile([P, 5 * CHB, P], dt.float8e4, tag="ohv")
                    oh_eng = nc.sync if t % 2 == 0 else nc.scalar
                    oh_eng.dma_start(
                        ohv[:, : bs * CHB, :],
                        oh1[:, b0 * CHB * P : (b0 + bs) * CHB * P].rearrange(
                            "p (c j) -> p c j", j=P))
                    if lnum != 3:
                        dstage = stp.tile([P, 5, FW], dt.bfloat16, tag="dnst")
                    for k in range(bs):
                        j = b0 + k
                        agg = psa.tile([P, P], dt.float32, tag="agg")
                        first = True
                        for g in range(NCHUNK):
                            tg = gtiles[(g, t)]
                            for c in range(GCH[g]):
                                nc.tensor.matmul(
                                    agg[:], tg[:, GCH[g] * k + c, :],
                                    ohv[:, k * CHB + COFF[g] + c, :],
                                    start=first, stop=False)
                                first = False
                        nc.tensor.matmul(
                            agg[:], gs[:, k, :], ident_t[:],
                            start=False, stop=True)

                        if lnum == 1:
                            # ZT = W0^T @ agg_x[0:14]; relu; dense W1; epilogue
                            axs = rlp.tile([14, P], dt.float32, tag="axs")
                            nc.vector.tensor_copy(axs[:], agg[0:14, :])
                            zt = psx.tile([P, P], dt.float32, tag="zt")
                            nc.tensor.matmul(
                                zt[:], w0_t[:], axs[:], start=True, stop=True)
                            rT = rlp.tile([P, P], dt.float32, tag="rT")
                            nc.scalar.activation(
                                rT[:], zt[:],
                                mybir.ActivationFunctionType.Relu)
                            h_ps = psd.tile([P, FW], dt.float32, tag="dnps")
                            nc.tensor.matmul(
                                h_ps[:], rT[:], w1_t[:], start=True, stop=True)
                            nc.vector.tensor_scalar(
                                dstage[:, k, :], h_ps[:],
                                dinv2_t[:, j : j + 1], None,
                                mybir.AluOpType.mult)
                        elif lnum == 2:
                            rT = rlp.tile([P, P], dt.float32, tag="rT")
                            nc.scalar.activation(
                                rT[:], agg[:],
                                mybir.ActivationFunctionType.Relu)
                            h_ps = psd.tile([P, FW], dt.float32, tag="dnps")
                            nc.tensor.matmul(
                                h_ps[:], rT[:], w2p_t[:], start=True, stop=True)
                            nc.vector.tensor_scalar(
                                dstage[:, k, :], h_ps[:],
                                dinv2_t[:, j : j + 1], None,
                                mybir.AluOpType.mult)
                        else:
                            # L3: z3 = d_inv[dst]*agg; relu; sum-pool by col
                            r3w = rlp.tile([32, P], dt.float32, tag="r3w")
                            nc.vector.tensor_tensor(
                                r3w[:], agg[0:32, :],
                                dinvb_t[:, j * P : (j + 1) * P],
                                mybir.AluOpType.mult)
                            r3 = rlp.tile([32, P], dt.float32, tag="r3")
                            nc.scalar.activation(
                                r3[:], r3w[:],
                                mybir.ActivationFunctionType.Relu,
                                accum_out=pooled_cols[:, j : j + 1])
                    for g in range(NCHUNK):
                        gtiles.pop((g, t), None)
                    if lnum != 3:
                        r0 = b0 * P
                        nc.sync.dma_start(
                            h_shard[r0 : r0 + bs * P, :].rearrange(
                                "(g p) f -> p g f", p=P),
                            dstage[:, :bs, :])
                        if t in AG_FIRE:
                            pending_ag = AG_FIRE[t]

            xg_srcs = [xg[SEG_BASE[g] : SEG_BASE[g + 1], :]
                       for g in range(NCHUNK)]
            g2_srcs = [g2_c[g][:, :] for g in range(NCHUNK)]
            g3_srcs = [g3_c[g][:, :] for g in range(NCHUNK)]
            # L1 (x-aggregation)
            layer(1, xg_srcs, xgo, g2s_t, g2_c, None)
            # L2 (fires the deferred last AllGather chunk of the L1
            # boundary); its own boundary only carries the 32 used columns
            layer(2, g2_srcs, g2s_t, g3s_t, g3_c,
                  lambda: ag_chunk(g2s_t, g2_c, 3), ag_w=32)
            # L3 + pooling
            layer(3, g3_srcs, g3s_t, None, None,
                  lambda: ag_chunk(g3s_t, g3_c, 3, 32))
            pooled = msc.tile([32, 1], dt.float32)
            nc.vector.tensor_reduce(
                pooled[:], pooled_cols[:],
                axis=mybir.AxisListType.X, op=mybir.AluOpType.add)

            # global pool AllReduce + MLP head (replicated)
            nc.sync.dma_start(pool_in[:], pooled[:])
            nc.gpsimd.collective_compute(
                "AllReduce", mybir.AluOpType.add,
                replica_groups=[list(range(NCORES))],
                ins=[pool_in.opt()], outs=[pool_out.opt()])
            pooled_g = msc.tile([32, 1], dt.float32)
            nc.sync.dma_start(pooled_g[:], pool_out[:])
            ps16 = psp.tile([16, 1], dt.float32, tag="mlp")
            nc.tensor.matmul(ps16[:], fc11w_t[:], pooled_g[:], start=True, stop=True)
            a16 = msc.tile([16, 1], dt.float32)
            nc.scalar.activation(
                a16[:], ps16[:], mybir.ActivationFunctionType.Relu,
                bias=fc11b_t[:])
            ps1 = psp.tile([1, 1], dt.float32, tag="mlp")
            nc.tensor.matmul(ps1[:], fc12w_t[:], a16[:], start=True, stop=True)
            o1 = msc.tile([1, 1], dt.float32)
            nc.scalar.activation(
                o1[:], ps1[:], mybir.ActivationFunctionType.Identity,
                bias=fc12b_t[:])
            nc.sync.dma_start(out[:], o1[:])

    nc.compile()
    return nc


def _get_nc():
    global _CACHED_NC
    if _CACHED_NC is None:
        _CACHED_NC = _build_kernel()
    return _CACHED_NC


def _make_in_maps(inputs):
    x = np.asarray(inputs["x"], np.float32)
    edge_index = np.asarray(inputs["edge_index"])
    xg, xg_own, idxs, oh1, dinv2, dinvb = _preprocess(x, edge_index)

    w2p = np.zeros((128, FW), np.float32)
    w2p[:, :32] = np.asarray(inputs["W2"], np.float32)
    common = {
        "xg": xg,
        "w0": np.asarray(inputs["W0"], np.float32),
        "w1": np.asarray(inputs["W1"], np.float32),
        "w2p": w2p,
        "fc11w": np.asarray(inputs["fc11_w"], np.float32),
        "fc11b": np.asarray(inputs["fc11_b"], np.float32).reshape(16, 1),
        "fc12w": np.asarray(inputs["fc12_w"], np.float32),
        "fc12b": np.asarray(inputs["fc12_b"], np.float32).reshape(1, 1),
        "ident": np.eye(P, dtype=BF16),
    }
    return [
        {**common, "xgo": np.ascontiguousarray(xg_own[c]), "idxs": idxs[c],
         "oh1": oh1[c].reshape(P, BPC * CHB * P),
         "dinv2": dinv2[c], "dinvb": dinvb[c]}
        for c in range(NCORES)
    ]


def run(trace=False, _inputs=None, **inputs):
    if _inputs is not None:
        inputs = _inputs
    in_maps = _make_in_maps(inputs)
    nc = _get_nc()
    res = run_bass_kernel_spmd(
        nc, in_maps, core_ids=list(range(NCORES)), trace=trace)
    y = np.asarray(res.results[0]["out"], np.float32).reshape(1)
    return y, res


def kernel(**inputs) -> np.ndarray:
    y, _ = run(**inputs)
    return y
